# revision 1
# baseline (speedup 1.0000x reference)
"""Trainium2 Bass kernel for nn_CrossAttention (AdaLN cross-attention).

Sharding: 8 cores = 4 batches x 2 q-halves (1024 q rows each). Each core
computes its full output rows; host gather is pure concatenation.

Shapes (full): x_q/x_k/single_cond_* [4,2048,128] f32, pair_logits
[4,4,2048,2048] f32, mask_q/mask_k [4,2048] bool. Output [4,2048,128] f32.
"""

import numpy as np

try:
    import concourse.bass as bass
except ImportError:  # stand-alone grading dir
    import sys

    sys.path.insert(0, "/opt/trn_rl_repo")
    import concourse.bass as bass

import concourse.mybir as mybir
import concourse.tile as tile
from concourse.bass_utils import run_bass_kernel_spmd
from concourse.masks import make_identity

F32 = mybir.dt.float32
F32R = mybir.dt.float32r
F16 = mybir.dt.float16
BF16 = mybir.dt.bfloat16
U8 = mybir.dt.uint8
AX = mybir.AxisListType.X
OP = mybir.AluOpType
AF = mybir.ActivationFunctionType

B, Q, K, D, H, DH = 4, 2048, 2048, 128, 4, 32
QH = 1024  # q rows per core
EPS = 1e-5
QSCALE = DH**-0.5
P = 128
NQB = QH // P  # 8 local q blocks
NKB = K // P  # 16 k blocks

TRACE = False
LAST_RESULT = None


def _ln_normalize(nc, pools, x_t, rows=P):
    """Return tile = (x - mean)/sqrt(var+eps) for [rows, D] tile (fp32)."""
    small, work = pools["small"], pools["work"]
    stats = small.tile([P, nc.vector.BN_STATS_DIM], F32, tag="stats")
    mv = small.tile([P, nc.vector.BN_AGGR_DIM], F32, tag="mv")
    nc.vector.bn_stats(out=stats[:rows], in_=x_t[:rows])
    nc.vector.bn_aggr(out=mv[:rows], in_=stats[:rows])
    negm = small.tile([P, 1], F32, tag="negm")
    nc.vector.tensor_scalar_mul(negm[:rows], mv[:rows, 0:1], -1.0)
    std = small.tile([P, 1], F32, tag="std")
    nc.scalar.activation(
        out=std[:rows], in_=mv[:rows, 1:2], func=AF.Sqrt, bias=pools["eps_col"][:rows]
    )
    rstd = small.tile([P, 1], F32, tag="rstd")
    nc.vector.reciprocal(out=rstd[:rows], in_=std[:rows])
    negmr = small.tile([P, 1], F32, tag="negmr")
    nc.vector.tensor_mul(out=negmr[:rows], in0=negm[:rows], in1=rstd[:rows])
    xn = work.tile([P, D], F32, tag="xn")
    nc.scalar.activation(
        out=xn[:rows], in_=x_t[:rows], func=AF.Identity,
        scale=rstd[:rows], bias=negmr[:rows],
    )
    return xn


def legalize_sync_waits(nc, max_waits=1):
    """This container's walrus rejects instructions carrying more than one
    sem wait ("Too many sync wait commands"). Split extra waits onto
    preceding same-engine NoOps — sequencers execute in order, so waiting
    immediately before the instruction is equivalent."""
    n = 0
    for fn in nc.m.functions:
        for blk in fn.blocks:
            out = []
            for inst in list(blk.instructions):
                si = getattr(inst, "sync_info", None)
                if si is not None and si.on_wait and len(si.on_wait) > max_waits:
                    waits = list(si.on_wait)
                    for j, w in enumerate(waits[:-max_waits]):
                        nop = mybir.InstNoOp(name=f"wsplit_{inst.name}_{j}")
                        nop.engine = inst.engine
                        nop.sync_info = mybir.SyncInfo(on_wait=[w], on_update=[])
                        out.append(nop)
                        n += 1
                    inst.sync_info = mybir.SyncInfo(
                        on_wait=waits[-max_waits:], on_update=list(si.on_update)
                    )
                out.append(inst)
            blk.instructions[:] = out
    return n


def build_nc(legalize=True):
    nc = bass.Bass("TRN2")

    # ---- I/O ----
    x_q = nc.dram_tensor("x_q", [QH, D], F32, kind="ExternalInput")
    cond_q = nc.dram_tensor("cond_q", [QH, D], F32, kind="ExternalInput")
    mask_q = nc.dram_tensor("mask_q", [QH], U8, kind="ExternalInput")
    x_k = nc.dram_tensor("x_k", [K, D], F32, kind="ExternalInput")
    cond_k = nc.dram_tensor("cond_k", [K, D], F32, kind="ExternalInput")
    mask_k = nc.dram_tensor("mask_k", [K], U8, kind="ExternalInput")
    pair = nc.dram_tensor("pair", [H, QH, K], F32, kind="ExternalInput")
    # ln_g already folded into scale/bias weights on host; bq pre-scaled by QSCALE
    wnames = [
        "q_scale_W", "q_scale_b", "q_bias_W",
        "k_scale_W", "k_scale_b", "k_bias_W",
        "Wq", "bq", "Wk", "Wv", "Wg", "Wt", "Wz", "bz",
    ]
    wshapes = {n: ([D] if n in ("q_scale_b", "k_scale_b", "bq", "bz") else [D, D]) for n in wnames}
    wd = {n: nc.dram_tensor(n, wshapes[n], F32, kind="ExternalInput") for n in wnames}
    y = nc.dram_tensor("y", [QH, D], F32, kind="ExternalOutput")

    with TileContext(nc) as tc:
        _body(nc, tc, x_q, cond_q, mask_q, x_k, cond_k, mask_k, pair, wd, y)
    if legalize:
        legalize_sync_waits(nc)
    return nc


from concourse.tile import TileContext  # noqa: E402


def _body(nc, tc, x_q, cond_q, mask_q, x_k, cond_k, mask_k, pair, wd, y):
    import contextlib

    ctx = contextlib.ExitStack()
    with ctx:
        consts = ctx.enter_context(tc.tile_pool(name="consts", bufs=1))
        persist = ctx.enter_context(tc.tile_pool(name="persist", bufs=1))
        small = ctx.enter_context(tc.tile_pool(name="small", bufs=8))
        work = ctx.enter_context(tc.tile_pool(name="work", bufs=3))
        pairp = ctx.enter_context(tc.tile_pool(name="pairp", bufs=6))
        wtp = ctx.enter_context(tc.tile_pool(name="wtp", bufs=6))
        wTp = ctx.enter_context(tc.tile_pool(name="wTp", bufs=2))
        eph = ctx.enter_context(tc.tile_pool(name="eph", bufs=2))
        ps_sm = ctx.enter_context(tc.tile_pool(name="ps_sm", bufs=2, space="PSUM"))
        dram = ctx.enter_context(tc.tile_pool(name="dram", bufs=1, space="DRAM"))
        pro_ctx = contextlib.ExitStack()
        pro = pro_ctx.enter_context(tc.tile_pool(name="pro", bufs=4, space="PSUM"))
        pro_sb = pro_ctx.enter_context(tc.tile_pool(name="pro_sb", bufs=1))

        pools = {"small": small, "work": work}

        # ---- constants ----
        ident = consts.tile([P, P], F32, tag="ident")
        make_identity(nc, ident)
        eps_col = consts.tile([P, 1], F32, tag="eps")
        nc.vector.memset(eps_col, EPS)
        pools["eps_col"] = eps_col
        ones_col = consts.tile([P, 1], F32, tag="ones_col")
        nc.vector.memset(ones_col, 1.0)
        ones_row = consts.tile([1, P], F32, tag="ones_row")
        nc.vector.memset(ones_row, 1.0)

        # weights
        wsb = {}
        for n in ("q_scale_W", "q_bias_W", "k_scale_W", "k_bias_W", "Wq", "Wk", "Wv", "Wg", "Wt", "Wz"):
            t = consts.tile([D, D], F32, tag=n)
            nc.sync.dma_start(out=t, in_=wd[n][:, :])
            wsb[n] = t
        cols = {}
        for n in ("q_scale_b", "k_scale_b", "bq", "bz"):
            t = consts.tile([D, 1], F32, tag=n)
            nc.sync.dma_start(out=t, in_=wd[n][:][:, None])
            cols[n] = t
        bqs = cols["bq"]  # pre-scaled by QSCALE on host

        # ---- masks ----
        # nmk [128,16] = 1 - mask_k, fp32 and fp16
        mku8 = small.tile([P, NKB], U8, tag="mku8")
        nc.gpsimd.dma_start(out=mku8, in_=mask_k[:].rearrange("(a b) -> b a", b=P))
        nmk_f = persist.tile([P, NKB], F32, tag="nmk_f")
        nc.vector.tensor_scalar(
            out=nmk_f, in0=mku8, scalar1=-1.0, scalar2=1.0, op0=OP.mult, op1=OP.add
        )
        nmk_h = persist.tile([P, NKB], F16, tag="nmk_h")
        nc.vector.tensor_copy(out=nmk_h, in_=nmk_f)

        # count of masked k, has-any, reciprocal
        ps_c = pro.tile([P, 512], F32, tag="pro")
        nc.tensor.matmul(ps_c[0:1, 0:NKB], lhsT=ones_col, rhs=nmk_f, start=True, stop=True)
        cnt = small.tile([1, 1], F32, tag="cnt")
        nc.vector.reduce_sum(out=cnt, in_=ps_c[0:1, 0:NKB], axis=AX)
        hr2 = small.tile([1, 2], F32, tag="hr2")
        nc.vector.tensor_scalar_min(hr2[:, 0:1], cnt, 1.0)
        sc = small.tile([1, 1], F32, tag="sc")
        nc.vector.tensor_scalar_max(sc, cnt, 1.0)
        nc.vector.reciprocal(out=hr2[:, 1:2], in_=sc)
        ps_b = pro.tile([P, 512], F32, tag="pro")
        nc.tensor.matmul(ps_b[:, 0:2], lhsT=ones_row, rhs=hr2, start=True, stop=True)
        hr_cols = persist.tile([P, 2], F32, tag="hr_cols")
        nc.any.tensor_copy(out=hr_cols, in_=ps_b[:, 0:2])
        has_col, rc_col = hr_cols[:, 0:1], hr_cols[:, 1:2]
        negh = persist.tile([P, 1], F32, tag="negh")
        nc.vector.tensor_scalar_mul(negh, has_col, -1.0)

        # nmq_b [128, 1024] = (1-mask_q(q)) * has, per free position q
        mqa = mask_q[:]
        mq_bc = bass.AP(tensor=mqa.tensor, offset=mqa.offset, ap=[[0, P]] + list(mqa.ap))
        mq_u8 = work.tile([P, QH], U8, tag="mq_u8")
        nc.gpsimd.dma_start(out=mq_u8, in_=mq_bc)
        nmq_b = persist.tile([P, QH], F32, tag="nmq_b")
        nc.scalar.activation(
            out=nmq_b, in_=mq_u8, func=AF.Identity, scale=negh, bias=has_col
        )

        # ---- AdaLN + transposes ----
        def transpose_into(dst, src_t, col0):
            pst = pro.tile([P, 512], F32, tag="pro")
            nc.tensor.transpose(pst[:, 0:P], src_t, ident)
            nc.any.tensor_copy(out=dst[:, col0 : col0 + P], in_=pst[:, 0:P])

        xnqT = pro_sb.tile([D, QH], F32, tag="xnqT")
        cnqT = pro_sb.tile([D, QH], F32, tag="cnqT")
        condqT = persist.tile([D, QH], F32, tag="condqT")
        xnkT = pro_sb.tile([D, K], F32, tag="xnkT")
        cnkT = pro_sb.tile([D, K], F32, tag="cnkT")

        for i in range(NQB):
            sl = slice(i * P, (i + 1) * P)
            xt = work.tile([P, D], F32, tag="xt")
            nc.sync.dma_start(out=xt, in_=x_q[sl, :])
            ct = work.tile([P, D], F32, tag="ct")
            nc.sync.dma_start(out=ct, in_=cond_q[sl, :])
            xn = _ln_normalize(nc, pools, xt)
            cn = _ln_normalize(nc, pools, ct)
            transpose_into(xnqT, xn, i * P)
            transpose_into(cnqT, cn, i * P)
            transpose_into(condqT, ct, i * P)
        for i in range(NKB):
            sl = slice(i * P, (i + 1) * P)
            xt = work.tile([P, D], F32, tag="xt")
            nc.sync.dma_start(out=xt, in_=x_k[sl, :])
            ct = work.tile([P, D], F32, tag="ct")
            nc.sync.dma_start(out=ct, in_=cond_k[sl, :])
            xn = _ln_normalize(nc, pools, xt)
            cn = _ln_normalize(nc, pools, ct)
            transpose_into(xnkT, xn, i * P)
            transpose_into(cnkT, cn, i * P)

        # adaln combine (T land): xT = sigmoid(scale_W.T@cnT + scale_b) * xnT + bias_W.T@cnT
        def adaln_T(cnT, xnT, n, side):
            out = pro_sb.tile([D, n], F32, tag=f"adaln_{side}")
            for j in range(n // 512):
                sl = slice(j * 512, (j + 1) * 512)
                ps1 = pro.tile([P, 512], F32, tag="pro")
                nc.tensor.matmul(ps1, lhsT=wsb[f"{side}_scale_W"], rhs=cnT[:, sl], start=True, stop=True)
                sig = eph.tile([D, 512], F32, tag="sig")
                nc.scalar.activation(out=sig, in_=ps1, func=AF.Sigmoid, bias=cols[f"{side}_scale_b"])
                ps2 = pro.tile([P, 512], F32, tag="pro")
                nc.tensor.matmul(ps2, lhsT=wsb[f"{side}_bias_W"], rhs=cnT[:, sl], start=True, stop=True)
                nc.vector.tensor_mul(out=out[:, sl], in0=sig, in1=xnT[:, sl])
                nc.vector.tensor_add(out=out[:, sl], in0=out[:, sl], in1=ps2)
            return out

        xqT = adaln_T(cnqT, xnqT, QH, "q")
        xkT = adaln_T(cnkT, xnkT, K, "k")

        # ---- projections ----
        qhT = persist.tile([D, QH], BF16, tag="qhT")
        for j in range(QH // 512):
            sl = slice(j * 512, (j + 1) * 512)
            ps1 = pro.tile([P, 512], F32, tag="pro")
            nc.tensor.matmul(ps1, lhsT=wsb["Wq"], rhs=xqT[:, sl], start=True, stop=True)
            nc.scalar.activation(out=qhT[:, sl], in_=ps1, func=AF.Identity, bias=bqs, scale=QSCALE)
        khT = persist.tile([D, K], BF16, tag="khT")
        for j in range(K // 512):
            sl = slice(j * 512, (j + 1) * 512)
            ps1 = pro.tile([P, 512], F32, tag="pro")
            nc.tensor.matmul(ps1, lhsT=wsb["Wk"], rhs=xkT[:, sl], start=True, stop=True)
            nc.any.tensor_copy(out=khT[:, sl], in_=ps1)
        vh = persist.tile([P, NKB, D], F16, tag="vh")
        for kb in range(NKB):
            sl = slice(kb * P, (kb + 1) * P)
            ps1 = pro.tile([P, 512], F32, tag="pro")
            nc.tensor.matmul(ps1[:, 0:D], lhsT=xkT[:, sl], rhs=wsb["Wv"], start=True, stop=True)
            nc.any.tensor_copy(out=vh[:, kb, :], in_=ps1[:, 0:D])
        # gate gT = sigmoid(Wg.T @ xqT)
        gT = persist.tile([D, QH], F32, tag="gT")
        for j in range(QH // 512):
            sl = slice(j * 512, (j + 1) * 512)
            ps1 = pro.tile([P, 512], F32, tag="pro")
            nc.tensor.matmul(ps1, lhsT=wsb["Wg"], rhs=xqT[:, sl], start=True, stop=True)
            nc.scalar.activation(out=gT[:, sl], in_=ps1, func=AF.Sigmoid)

        # masked-row value: M_col[c] = sum_k nmk[k] vh[k,c] / count
        ps_m = pro.tile([P, 512], F32, tag="pro")
        for h in range(H):
            for kb in range(NKB):
                nc.tensor.matmul(
                    ps_m[h * DH : (h + 1) * DH, 0:1],
                    lhsT=vh[:, kb, h * DH : (h + 1) * DH],
                    rhs=nmk_h[:, kb : kb + 1],
                    start=(kb == 0),
                    stop=(kb == NKB - 1),
                    tile_position=(0, h * DH),
                )
        M_col = persist.tile([P, 1], F32, tag="M_col")
        nc.scalar.activation(
            out=M_col, in_=ps_m[:, 0:1], func=AF.Identity, scale=rc_col
        )

        # release prologue psum space; open main-loop psum pools
        pro_ctx.close()
        ps_l = ctx.enter_context(tc.tile_pool(name="ps_l", bufs=2, space="PSUM"))
        ps_av = ctx.enter_context(tc.tile_pool(name="ps_av", bufs=1, space="PSUM"))

        # ---- main attention loop ----
        s_drams = [
            dram.tile([H, 512], F32, tag=f"s_dram{half}", name=f"s_dram{half}")
            for half in range(2)
        ]
        y_view = y  # [QH, D]
        pair_v = pair  # [H, QH, K]

        for half in range(2):
            qbs = range(half * 4, (half + 1) * 4)
            psA = ps_av.tile([P, 512], F32, tag="psA", name=f"psA{half}")
            for h in range(H):
                # per qb: logits, exp, sums
                wt_tiles = {}
                for qb in qbs:
                    qsl = slice(qb * P, (qb + 1) * P)
                    pair_sb = pairp.tile([P, K], F32, tag="pair", name=f"pair{half}{h}{qb}")
                    nc.sync.dma_start(out=pair_sb, in_=pair_v[h, qsl, :])
                    pair_r = pair_sb.rearrange("p (a c) -> p a c", c=512)
                    wt = wtp.tile([P, K], F16, tag="wt", name=f"wt{half}{h}{qb}")
                    wt_r = wt.rearrange("p (a c) -> p a c", c=512)
                    saccs = []
                    for kh in range(2):
                        psl = ps_l.tile([P, 2, 512], F32, tag="psl", name=f"psl{half}{h}{qb}{kh}")
                        for ks in range(2):
                            ksl = slice((kh * 2 + ks) * 512, (kh * 2 + ks + 1) * 512)
                            nc.tensor.matmul(
                                psl[:, ks, :],
                                lhsT=qhT[h * DH : (h + 1) * DH, qsl],
                                rhs=khT[h * DH : (h + 1) * DH, ksl],
                                start=True,
                                stop=True,
                                tile_position=(h * DH, 0),
                            )
                        nc.vector.tensor_add(
                            out=psl, in0=psl, in1=pair_r[:, kh * 2 : kh * 2 + 2, :]
                        )
                        sacc = small.tile([P, 1], F32, tag="sacc", name=f"sacc{half}{h}{qb}{kh}")
                        nc.scalar.activation(
                            out=wt_r[:, kh * 2 : kh * 2 + 2, :],
                            in_=psl,
                            func=AF.Exp,
                            accum_out=sacc,
                        )
                        saccs.append(sacc)
                    rs = small.tile([P, 1], F32, tag="rs", name=f"rs{half}{h}{qb}")
                    nc.vector.tensor_add(out=rs, in0=saccs[0], in1=saccs[1])
                    nc.vector.reciprocal(out=rs, in_=rs)
                    nc.gpsimd.dma_start(
                        out=s_drams[half][h, (qb - half * 4) * P : (qb - half * 4 + 1) * P],
                        in_=rs,
                    )
                    wt_tiles[qb] = wt

                # transpose weights (fp16 xbar; one call per qb writes a
                # strided [128,16,128] slice: out[p,m,q] = wt[q, m*128+p])
                # then AV matmul accumulating over kb
                wT = wTp.tile([P, NKB, 512], F16, tag="wT", name=f"wT{half}{h}")
                for qb in qbs:
                    ql = qb - half * 4
                    nc.scalar.dma_start(
                        out=wT[:, :, ql * P : (ql + 1) * P],
                        in_=wt_tiles[qb],
                        transpose=True,
                    )
                for kb in range(NKB):
                    nc.tensor.matmul(
                        psA[h * DH : (h + 1) * DH, :],
                        lhsT=vh[:, kb, h * DH : (h + 1) * DH],
                        rhs=wT[:, kb, :],
                        start=(kb == 0),
                        stop=(kb == NKB - 1),
                        tile_position=(0, h * DH),
                    )

            # read back 1/s broadcast to [128c, 512q]
            sda = s_drams[half][:, :]
            rs_bc = bass.AP(
                tensor=sda.tensor,
                offset=sda.offset,
                ap=[[512, H], [0, DH], [1, 512]],
            )
            rsT = eph.tile([P, 512], F32, tag="rsT", name=f"rsT{half}")
            nc.gpsimd.dma_start(out=rsT, in_=rs_bc)

            # normalize, blend masked rows, gate, project, final sigmoid gate
            qsl512 = slice(half * 512, (half + 1) * 512)
            outT = eph.tile([P, 512], F32, tag="outT")
            nc.vector.tensor_mul(out=outT, in0=psA, in1=rsT)
            tmp = eph.tile([P, 512], F32, tag="tmp")
            nc.vector.tensor_sub(out=tmp, in0=M_col.to_broadcast([P, 512]), in1=outT)
            nc.vector.tensor_mul(out=tmp, in0=tmp, in1=nmq_b[:, qsl512])
            nc.vector.tensor_add(out=outT, in0=outT, in1=tmp)
            nc.vector.tensor_mul(out=outT, in0=outT, in1=gT[:, qsl512])
            ps_z = ps_sm.tile([P, 512], F32, tag="psmall")
            nc.tensor.matmul(ps_z, lhsT=wsb["Wt"], rhs=outT, start=True, stop=True)
            ps_s = ps_sm.tile([P, 512], F32, tag="psmall")
            nc.tensor.matmul(ps_s, lhsT=wsb["Wz"], rhs=condqT[:, qsl512], start=True, stop=True)
            szT = eph.tile([P, 512], F32, tag="szT")
            nc.scalar.activation(out=szT, in_=ps_s, func=AF.Sigmoid, bias=cols["bz"])
            yT = eph.tile([P, 512], F32, tag="yT")
            nc.vector.tensor_mul(out=yT, in0=ps_z, in1=szT)
            for j in range(4):
                qb = half * 4 + j
                pst = ps_sm.tile([P, 512], F32, tag="psmall")
                nc.tensor.transpose(pst[:, 0:P], yT[:, j * P : (j + 1) * P], ident)
                ysb = eph.tile([P, D], F32, tag="ysb")
                nc.any.tensor_copy(out=ysb, in_=pst[:, 0:P])
                nc.sync.dma_start(out=y_view[qb * P : (qb + 1) * P, :], in_=ysb)


_NC_CACHE = None


def _get_nc():
    global _NC_CACHE
    if _NC_CACHE is None:
        _NC_CACHE = build_nc()
    return _NC_CACHE


def kernel(**inputs):
    global LAST_RESULT
    nc = _get_nc()
    xq = np.ascontiguousarray(inputs["x_q"], dtype=np.float32)
    xk = np.ascontiguousarray(inputs["x_k"], dtype=np.float32)
    cq = np.ascontiguousarray(inputs["single_cond_q"], dtype=np.float32)
    ck = np.ascontiguousarray(inputs["single_cond_k"], dtype=np.float32)
    mq = np.ascontiguousarray(inputs["mask_q"]).astype(np.uint8)
    mk = np.ascontiguousarray(inputs["mask_k"]).astype(np.uint8)
    pl = np.ascontiguousarray(inputs["pair_logits"], dtype=np.float32)
    wn = [
        "q_scale_W", "q_scale_b", "q_bias_W",
        "k_scale_W", "k_scale_b", "k_bias_W",
        "Wq", "bq", "Wk", "Wv", "Wg", "Wt", "Wz", "bz",
    ]
    wmaps = {n: np.ascontiguousarray(inputs[n], dtype=np.float32) for n in wn}
    # fold ln gamma into the adaln projection weights; pre-scale bq
    for side in ("q", "k"):
        g = np.asarray(inputs[f"{side}_ln_g"], np.float32)[:, None]
        wmaps[f"{side}_scale_W"] = np.ascontiguousarray(g * wmaps[f"{side}_scale_W"])
        wmaps[f"{side}_bias_W"] = np.ascontiguousarray(g * wmaps[f"{side}_bias_W"])
    wmaps["bq"] = np.ascontiguousarray(wmaps["bq"] * QSCALE)

    in_maps = []
    for c in range(8):
        b, qh = c // 2, c % 2
        qsl = slice(qh * QH, (qh + 1) * QH)
        m = {
            "x_q": xq[b, qsl],
            "cond_q": cq[b, qsl],
            "mask_q": mq[b, qsl],
            "x_k": xk[b],
            "cond_k": ck[b],
            "mask_k": mk[b],
            "pair": np.ascontiguousarray(pl[b, :, qsl, :]),
        }
        m.update(wmaps)
        in_maps.append(m)

    res = run_bass_kernel_spmd(nc, in_maps, core_ids=list(range(8)), trace=TRACE)
    LAST_RESULT = res
    out = np.empty((B, Q, D), np.float32)
    for c in range(8):
        b, qh = c // 2, c % 2
        out[b, qh * QH : (qh + 1) * QH, :] = res.results[c]["y"]
    return out

